# revision 3
# baseline (speedup 1.0000x reference)
"""GAT (2-layer, CORA-style) on 8 Trainium2 NeuronCores.

Strategy (per spec sharding_hint): nodes are dst-sharded across 8 cores.
  Phase 1: each core computes h = x@W1 (+ attention scores) for its 12.5K
           nodes from a host-pre-transposed bf16 x shard, writes a packed
           fp16 per-node table row [h(64) | a_src(8)] to DRAM.
  AllGather: table shards -> full table (+ sentinel row for padding).
  Edge phase: per dst-tile of 128 degree-sorted dsts x D slots, gather all
           P*D src rows with a SINGLE batched indirect DMA (the offset AP
           carries the whole [P, D] index tile; one SWDGE op per tile
           instead of D, amortizing the ~1us fixed descriptor-gen cost),
           segment-softmax per partition row (max-free: exp(lrelu(s)-8),
           shift cancels in normalization), messages reduced along the
           slot (free) axis.
  Phase 2: tiny fused matmul [W2 | W2@att_src2 | W2@att_dst2], second
           table AllGather, second batched-gather edge phase, log_softmax,
           output shard.

Host work is limited to sharding/layout prep: permutations, transposes,
CSR/padding of the (input-given) edge list, and parameter repacking.
"""

import sys

sys.path.insert(0, "/opt/trn_rl_repo")

import numpy as np
import ml_dtypes

BF16 = ml_dtypes.bfloat16


def _default_cfg():
    return dict(N=100000, E=3200000, F_IN=1433, H1=8, HID=8, NCLS=7, NC=8)


# --------------------------------------------------------------------------
# Host-side prep: sharding, degree-sort permutation, padded CSR tiles.
# --------------------------------------------------------------------------
def host_prep(inputs, cfg):
    N, NC, F_IN = cfg["N"], cfg["NC"], cfg["F_IN"]
    H1, HID, NCLS = cfg["H1"], cfg["HID"], cfg["NCLS"]
    NSH = N // NC
    x = np.asarray(inputs["x"], dtype=np.float32)
    ei = np.asarray(inputs["edge_index"], dtype=np.int64)
    W1 = np.asarray(inputs["W1"], dtype=np.float32)
    as1 = np.asarray(inputs["att_src1"], dtype=np.float32)
    ad1 = np.asarray(inputs["att_dst1"], dtype=np.float32)
    b1 = np.asarray(inputs["b1"], dtype=np.float32)
    W2 = np.asarray(inputs["W2"], dtype=np.float32)
    as2 = np.asarray(inputs["att_src2"], dtype=np.float32)
    ad2 = np.asarray(inputs["att_dst2"], dtype=np.float32)
    b2 = np.asarray(inputs["b2"], dtype=np.float32)

    loops = np.arange(N, dtype=np.int64)
    src_all = np.concatenate([ei[0], loops])
    dst_all = np.concatenate([ei[1], loops])

    deg = np.bincount(dst_all, minlength=N)

    # per-core degree-descending permutation of the core's own nodes
    gid = np.empty(N, dtype=np.int64)  # orig node id -> permuted global id
    perm_nodes = []  # per core: orig node ids in permuted order
    for c in range(NC):
        lo = c * NSH
        order = np.argsort(-deg[lo : lo + NSH], kind="stable")
        perm_nodes.append(lo + order)
        pos = np.empty(NSH, dtype=np.int64)
        pos[order] = np.arange(NSH)
        gid[lo : lo + NSH] = lo + pos

    pd = gid[dst_all]
    gs = gid[src_all].astype(np.int32)
    eorder = np.argsort(pd, kind="stable")
    gss = gs[eorder]
    counts = np.bincount(pd, minlength=N)
    row_start = np.zeros(N + 1, dtype=np.int64)
    np.cumsum(counts, out=row_start[1:])

    # tile schedule (shared across cores for SPMD)
    P_list = []
    t0 = 0
    while t0 < NSH:
        P_list.append(min(128, NSH - t0))
        t0 += 128
    NT = len(P_list)
    D_list = []
    for t in range(NT):
        # degree-sorted desc -> first row of the tile has the max degree
        D_list.append(int(max(counts[c * NSH + t * 128] for c in range(NC))))
    D_list = [max(d, 1) for d in D_list]

    # padded index matrices per core (pad -> sentinel row N)
    idx_cores = []
    for c in range(NC):
        parts = []
        for t in range(NT):
            P, D = P_list[t], D_list[t]
            g0 = c * NSH + t * 128
            L = counts[g0 : g0 + P]
            seg = gss[row_start[g0] : row_start[g0 + P]]
            mat = np.full((P, D), N, dtype=np.int32)
            mask = np.arange(D)[None, :] < L[:, None]
            mat[mask] = seg
            parts.append(mat.ravel())
        idx_cores.append(np.ascontiguousarray(np.concatenate(parts), dtype=np.int32))
    TOT = idx_cores[0].size
    assert all(a.size == TOT for a in idx_cores)

    # x shards, permuted + transposed, bf16
    xts = [np.ascontiguousarray(x[perm_nodes[c]].T).astype(BF16) for c in range(NC)]

    # params
    attm = np.zeros((H1 * HID, 2 * H1), dtype=np.float32)
    for h in range(H1):
        attm[h * HID : (h + 1) * HID, h] = as1[h]
        attm[h * HID : (h + 1) * HID, H1 + h] = ad1[h]
    m2 = np.zeros((HID, NCLS + 2), dtype=np.float32)
    m2[:, :NCLS] = W2
    m2[:, NCLS] = W2 @ as2[0]
    m2[:, NCLS + 1] = W2 @ ad2[0]
    b1r = np.ascontiguousarray(np.tile(b1[None, :], (128, 1)), dtype=np.float32)
    b2r = np.ascontiguousarray(np.tile(b2[None, :], (128, 1)), dtype=np.float32)
    ROW1 = H1 * HID + H1  # 72
    sent1 = np.zeros((1, ROW1), dtype=np.float16)
    sent1[0, H1 * HID :] = -60.0
    sent2 = np.zeros((1, NCLS + 1), dtype=np.float16)
    sent2[0, NCLS] = -60.0
    ident = np.ascontiguousarray(np.eye(128, dtype=np.float32))

    meta = dict(
        cfg=cfg,
        NSH=NSH,
        NT=NT,
        P_list=P_list,
        D_list=D_list,
        TOT=TOT,
        ROW1=ROW1,
    )
    in_maps = []
    for c in range(NC):
        in_maps.append(
            dict(
                xt=xts[c],
                w1=W1.astype(BF16),
                attm=attm.astype(BF16),
                m2=m2.astype(BF16),
                b1r=b1r,
                b2r=b2r,
                sent1=sent1,
                sent2=sent2,
                ident=ident,
                identb=ident.astype(BF16),
                idx=idx_cores[c],
            )
        )
    return meta, in_maps, perm_nodes


# --------------------------------------------------------------------------
# Bass program
# --------------------------------------------------------------------------
def build_program(meta):
    from concourse import bacc, bass, tile, mybir

    cfg = meta["cfg"]
    N, NC, F_IN = cfg["N"], cfg["NC"], cfg["F_IN"]
    H1, HID, NCLS = cfg["H1"], cfg["HID"], cfg["NCLS"]
    NSH, NT = meta["NSH"], meta["NT"]
    P_list, D_list, TOT = meta["P_list"], meta["D_list"], meta["TOT"]
    ROW1 = meta["ROW1"]  # 72
    ROW2 = NCLS + 1  # 8
    C1 = H1 * HID  # 64
    f32, f16, i32 = mybir.dt.float32, mybir.dt.float16, mybir.dt.int32
    bf16 = mybir.dt.bfloat16
    AX = mybir.AxisListType.X
    OP = mybir.AluOpType
    AF = mybir.ActivationFunctionType

    nc = bacc.Bacc(
        "TRN2",
        target_bir_lowering=False,
        debug=False,
        num_devices=NC,
    )

    xt = nc.dram_tensor("xt", [F_IN, NSH], bf16, kind="ExternalInput").ap()
    w1 = nc.dram_tensor("w1", [F_IN, C1], bf16, kind="ExternalInput").ap()
    attm = nc.dram_tensor("attm", [C1, 2 * H1], bf16, kind="ExternalInput").ap()
    m2 = nc.dram_tensor("m2", [HID, NCLS + 2], bf16, kind="ExternalInput").ap()
    b1r = nc.dram_tensor("b1r", [128, HID], f32, kind="ExternalInput").ap()
    b2r = nc.dram_tensor("b2r", [128, NCLS], f32, kind="ExternalInput").ap()
    sent1 = nc.dram_tensor("sent1", [1, ROW1], f16, kind="ExternalInput").ap()
    sent2 = nc.dram_tensor("sent2", [1, ROW2], f16, kind="ExternalInput").ap()
    ident = nc.dram_tensor("ident", [128, 128], f32, kind="ExternalInput").ap()
    identb = nc.dram_tensor("identb", [128, 128], bf16, kind="ExternalInput").ap()
    idx_h = nc.dram_tensor("idx", [TOT], i32, kind="ExternalInput")
    out = nc.dram_tensor("out", [NSH, NCLS], f32, kind="ExternalOutput").ap()

    # k-tiles over F_IN, node chunks of 512
    ktl = []
    k0 = 0
    while k0 < F_IN:
        ktl.append((k0, min(128, F_IN - k0)))
        k0 += 128
    chunks = []
    n0 = 0
    while n0 < NSH:
        chunks.append((n0, min(512, NSH - n0)))
        n0 += 512

    def slices_of(c0, csz):
        out_s = []
        s0 = 0
        while s0 < csz:
            ssz = min(128, csz - s0)
            out_s.append((c0 + s0, ssz))
            s0 += ssz
        return out_s

    # tile -> column offset into the persistent index matrix
    off_list = []
    o = 0
    for t in range(NT):
        off_list.append(o)
        o += D_list[t]
    SUMD = o

    with tile.TileContext(nc) as tc:
        with (
            tc.tile_pool(name="dram", bufs=1, space="DRAM") as dpool,
            tc.tile_pool(name="consts", bufs=1) as cpool,
            tc.tile_pool(name="persist", bufs=1) as ppool,
            tc.tile_pool(name="xload", bufs=4) as xpool,
            tc.tile_pool(name="work", bufs=3) as wpool,
            tc.tile_pool(name="ps", bufs=2, space="PSUM") as pspool,
        ):
            # ---- DRAM internals
            tb1s = dpool.tile([NSH, ROW1], f16, tag="tb1s")
            tb1f = dpool.tile([N + 1, ROW1], f16, tag="tb1f")
            tb2s = dpool.tile([NSH, ROW2], f16, tag="tb2s")
            tb2f = dpool.tile([N + 1, ROW2], f16, tag="tb2f")

            # ---- constants to SBUF
            w1t = []
            for ki, (k0, ks) in enumerate(ktl):
                wt = cpool.tile([ks, C1], bf16, tag=f"w1_{ki}")
                nc.sync.dma_start(wt[:], w1[k0 : k0 + ks, :])
                w1t.append(wt)
            attm_sb = cpool.tile([C1, 2 * H1], bf16, tag="attm")
            nc.sync.dma_start(attm_sb[:], attm[:])
            m2_sb = cpool.tile([HID, NCLS + 2], bf16, tag="m2")
            nc.sync.dma_start(m2_sb[:], m2[:])
            b1_sb = cpool.tile([128, HID], f32, tag="b1")
            nc.sync.dma_start(b1_sb[:], b1r[:])
            b2_sb = cpool.tile([128, NCLS], f32, tag="b2")
            nc.sync.dma_start(b2_sb[:], b2r[:])
            id_sb = cpool.tile([128, 128], f32, tag="ident")
            nc.sync.dma_start(id_sb[:], ident[:])
            idb_sb = cpool.tile([128, 128], bf16, tag="identb")
            nc.sync.dma_start(idb_sb[:], identb[:])
            s1_sb = cpool.tile([1, ROW1], f16, tag="s1")
            nc.sync.dma_start(s1_sb[:], sent1[:])
            s2_sb = cpool.tile([1, ROW2], f16, tag="s2")
            nc.sync.dma_start(s2_sb[:], sent2[:])
            nbias = cpool.tile([128, 1], f32, tag="nbias")
            nc.gpsimd.memset(nbias[:], -8.0)
            # sentinel rows of the full tables
            nc.sync.dma_start(tb1f[N : N + 1, :], s1_sb[:])
            nc.sync.dma_start(tb2f[N : N + 1, :], s2_sb[:])

            # ---- persistent per-node locals (dst-sharded)
            ad1_all = ppool.tile([128, NT * H1], f32, tag="ad1_all")
            ad2_all = ppool.tile([128, NT], f32, tag="ad2_all")
            eluT = ppool.tile([HID, NSH], bf16, tag="eluT")
            idx_all = ppool.tile([128, SUMD], i32, tag="idx_all")

            # preload ALL per-tile gather indices (used by both edge phases)
            flat_off = 0
            for t in range(NT):
                P, D = P_list[t], D_list[t]
                nc.sync.dma_start(
                    idx_all[0:P, off_list[t] : off_list[t] + D],
                    bass.AP(idx_h, flat_off, [[D, P], [1, D]]),
                )
                flat_off += P * D

            # =========================== PHASE 1 ===========================
            for ch, (c0, csz) in enumerate(chunks):
                hp = pspool.tile([C1, csz], f32, tag="mm")
                for ki, (k0, ks) in enumerate(ktl):
                    xtt = xpool.tile([ks, csz], bf16, tag="xt")
                    nc.sync.dma_start(xtt[:], xt[k0 : k0 + ks, c0 : c0 + csz])
                    nc.tensor.matmul(
                        hp[:],
                        w1t[ki][:],
                        xtt[:],
                        start=(ki == 0),
                        stop=(ki == len(ktl) - 1),
                    )
                h_sb = wpool.tile([C1, csz], bf16, tag="h_sb")
                nc.vector.tensor_copy(h_sb[:], hp[:])
                sp = pspool.tile([2 * H1, csz], f32, tag="sc")
                nc.tensor.matmul(sp[:], attm_sb[:], h_sb[:], start=True, stop=True)
                sc_sb = wpool.tile([2 * H1, csz], f32, tag="sc_sb")
                nc.vector.tensor_copy(sc_sb[:], sp[:])
                for s0, ssz in slices_of(c0, csz):
                    t_idx = s0 // 128
                    sl = slice(s0 - c0, s0 - c0 + ssz)
                    tp = pspool.tile([ssz, C1], f32, tag="tr")
                    nc.tensor.transpose(tp[:], h_sb[:, sl], idb_sb[:C1, :C1])
                    st = pspool.tile([ssz, 2 * H1], f32, tag="tr2")
                    nc.tensor.transpose(st[:], sc_sb[:, sl], id_sb[: 2 * H1, : 2 * H1])
                    row = wpool.tile([ssz, ROW1], f16, tag="row")
                    nc.scalar.activation(row[:, 0:C1], tp[:], AF.Copy)
                    nc.vector.tensor_copy(row[:, C1:ROW1], st[:, 0:H1])
                    nc.vector.tensor_copy(
                        ad1_all[0:ssz, t_idx * H1 : (t_idx + 1) * H1],
                        st[:, H1 : 2 * H1],
                    )
                    nc.sync.dma_start(tb1s[s0 : s0 + ssz, :], row[:])

            # ---- AllGather table 1
            nc.gpsimd.collective_compute(
                "AllGather",
                OP.bypass,
                replica_groups=[list(range(NC))],
                ins=[tb1s[:].opt()],
                outs=[tb1f[0:N, :].opt()],
            )

            # =========================== EDGE PHASE 1 ======================
            for t in range(NT):
                P, D = P_list[t], D_list[t]
                it = idx_all[0:P, off_list[t] : off_list[t] + D]
                G = wpool.tile([P, D * ROW1], f16, tag="G")
                G3 = G[:].rearrange("p (d v) -> p d v", d=D)
                nc.gpsimd.indirect_dma_start(
                    out=G[:],
                    out_offset=None,
                    in_=tb1f[:],
                    in_offset=bass.IndirectOffsetOnAxis(ap=it, axis=0),
                )
                # s = a_src[src] + a_dst[dst]
                sT = wpool.tile([P, D * H1], f32, tag="sT")
                sT3 = sT[:].rearrange("p (d h) -> p d h", d=D)
                ad_b = (
                    ad1_all[0:P, t * H1 : (t + 1) * H1]
                    .unsqueeze(1)
                    .broadcast_to([P, D, H1])
                )
                nc.vector.tensor_tensor(sT3, G3[:, :, C1:ROW1], ad_b, OP.add)
                s02 = wpool.tile([P, D * H1], f32, tag="s02")
                nc.vector.tensor_scalar(s02[:], sT[:], 0.2, None, op0=OP.mult)
                eT = wpool.tile([P, D * H1], f32, tag="eT")
                nc.vector.tensor_tensor(eT[:], sT[:], s02[:], OP.max)
                pT = wpool.tile([P, D * H1], f16, tag="pT")
                nc.scalar.activation(pT[:], eT[:], AF.Exp, bias=nbias[0:P, :])
                den = wpool.tile([P, H1], f32, tag="den")
                nc.vector.tensor_reduce(
                    den[:],
                    pT[:].rearrange("p (d h) -> p h d", d=D),
                    axis=AX,
                    op=OP.add,
                )
                rr = wpool.tile([P, H1], f32, tag="rr")
                nc.vector.reciprocal(rr[:], den[:])
                rr2 = wpool.tile([P, H1], f32, tag="rr2")
                nc.vector.tensor_scalar(rr2[:], rr[:], 1.0 / H1, None, op0=OP.mult)
                # messages
                m1 = wpool.tile([P, D * C1], f16, tag="m1")
                m1v = m1[:].rearrange("p (d h c) -> p d h c", d=D, h=H1)
                Ghv = G3[:, :, 0:C1].rearrange("p d (h c) -> p d h c", h=H1)
                p_b = (
                    pT[:]
                    .rearrange("p (d h) -> p d h", d=D)
                    .unsqueeze(3)
                    .broadcast_to([P, D, H1, HID])
                )
                nc.vector.tensor_tensor(m1v, Ghv, p_b, OP.mult)
                u = wpool.tile([P, C1], f32, tag="u")
                nc.vector.tensor_reduce(
                    u[:],
                    m1[:].rearrange("p (d q) -> p q d", d=D),
                    axis=AX,
                    op=OP.add,
                )
                o1 = wpool.tile([P, C1], f32, tag="o1")
                r_b = rr2[:].unsqueeze(2).broadcast_to([P, H1, HID])
                nc.vector.tensor_tensor(
                    o1[:].rearrange("p (h c) -> p h c", h=H1),
                    u[:].rearrange("p (h c) -> p h c", h=H1),
                    r_b,
                    OP.mult,
                )
                om = wpool.tile([P, HID], f32, tag="om")
                nc.vector.tensor_reduce(
                    om[:],
                    o1[:].rearrange("p (h c) -> p c h", h=H1),
                    axis=AX,
                    op=OP.add,
                )
                om2 = wpool.tile([P, HID], f32, tag="om2")
                nc.vector.tensor_tensor(om2[:], om[:], b1_sb[0:P, :], OP.add)
                # elu
                mn = wpool.tile([P, HID], f32, tag="mn")
                nc.vector.tensor_scalar(mn[:], om2[:], 0.0, None, op0=OP.min)
                ex = wpool.tile([P, HID], f32, tag="ex")
                nc.scalar.activation(ex[:], mn[:], AF.Exp)
                rl = wpool.tile([P, HID], f32, tag="rl")
                nc.vector.tensor_scalar(rl[:], om2[:], 0.0, None, op0=OP.max)
                ex2 = wpool.tile([P, HID], f32, tag="ex2")
                nc.vector.tensor_scalar(ex2[:], ex[:], -1.0, None, op0=OP.add)
                elu = wpool.tile([P, HID], f32, tag="elu")
                nc.vector.tensor_tensor(elu[:], rl[:], ex2[:], OP.add)
                # fused transpose into eluT (feeds phase-2 matmul)
                ep = pspool.tile([HID, P], f32, tag="sc")
                nc.tensor.transpose(ep[:], elu[:], id_sb[:P, :P])
                nc.scalar.activation(
                    eluT[:, t * 128 : t * 128 + P], ep[:], AF.Copy
                )

            # =========================== PHASE 2 ===========================
            for ch, (c0, csz) in enumerate(chunks):
                o2p = pspool.tile([NCLS + 2, csz], f32, tag="mm")
                nc.tensor.matmul(
                    o2p[:], m2_sb[:], eluT[:, c0 : c0 + csz], start=True, stop=True
                )
                o2_sb = wpool.tile([NCLS + 2, csz], f32, tag="o2_sb")
                nc.vector.tensor_copy(o2_sb[:], o2p[:])
                for s0, ssz in slices_of(c0, csz):
                    t_idx = s0 // 128
                    sl = slice(s0 - c0, s0 - c0 + ssz)
                    tp2 = pspool.tile([ssz, NCLS + 2], f32, tag="tr")
                    nc.tensor.transpose(
                        tp2[:], o2_sb[:, sl], id_sb[: NCLS + 2, : NCLS + 2]
                    )
                    row2 = wpool.tile([ssz, ROW2], f16, tag="row2")
                    nc.scalar.activation(row2[:], tp2[:, 0:ROW2], AF.Copy)
                    nc.vector.tensor_copy(
                        ad2_all[0:ssz, t_idx : t_idx + 1], tp2[:, ROW2 : ROW2 + 1]
                    )
                    nc.sync.dma_start(tb2s[s0 : s0 + ssz, :], row2[:])

            nc.gpsimd.collective_compute(
                "AllGather",
                OP.bypass,
                replica_groups=[list(range(NC))],
                ins=[tb2s[:].opt()],
                outs=[tb2f[0:N, :].opt()],
            )

            # =========================== EDGE PHASE 2 ======================
            for t in range(NT):
                P, D = P_list[t], D_list[t]
                it = idx_all[0:P, off_list[t] : off_list[t] + D]
                G2 = wpool.tile([P, D * ROW2], f16, tag="G2")
                G23 = G2[:].rearrange("p (d v) -> p d v", d=D)
                nc.gpsimd.indirect_dma_start(
                    out=G2[:],
                    out_offset=None,
                    in_=tb2f[:],
                    in_offset=bass.IndirectOffsetOnAxis(ap=it, axis=0),
                )
                s2t = wpool.tile([P, D], f32, tag="s2t")
                ad2_b = ad2_all[0:P, t : t + 1].broadcast_to([P, D])
                nc.vector.tensor_tensor(
                    s2t[:], G23[:, :, NCLS].squeeze(), ad2_b, OP.add
                )
                s202 = wpool.tile([P, D], f32, tag="s202")
                nc.vector.tensor_scalar(s202[:], s2t[:], 0.2, None, op0=OP.mult)
                e2t = wpool.tile([P, D], f32, tag="e2t")
                nc.vector.tensor_tensor(e2t[:], s2t[:], s202[:], OP.max)
                p2t = wpool.tile([P, D], f32, tag="p2t")
                nc.scalar.activation(p2t[:], e2t[:], AF.Exp)
                den2 = wpool.tile([P, 1], f32, tag="den2")
                nc.vector.tensor_reduce(den2[:], p2t[:], axis=AX, op=OP.add)
                r2 = wpool.tile([P, 1], f32, tag="r2")
                nc.vector.reciprocal(r2[:], den2[:])
                mm2 = wpool.tile([P, D * NCLS], f16, tag="mm2")
                p2_b = p2t[:].unsqueeze(2).broadcast_to([P, D, NCLS])
                nc.vector.tensor_tensor(
                    mm2[:].rearrange("p (d c) -> p d c", d=D),
                    G23[:, :, 0:NCLS],
                    p2_b,
                    OP.mult,
                )
                u2 = wpool.tile([P, NCLS], f32, tag="u2")
                nc.vector.tensor_reduce(
                    u2[:],
                    mm2[:].rearrange("p (d c) -> p c d", d=D),
                    axis=AX,
                    op=OP.add,
                )
                o2 = wpool.tile([P, NCLS], f32, tag="o2")
                nc.vector.tensor_scalar(o2[:], u2[:], r2[:], None, op0=OP.mult)
                o2b = wpool.tile([P, NCLS], f32, tag="o2b")
                nc.vector.tensor_tensor(o2b[:], o2[:], b2_sb[0:P, :], OP.add)
                # log_softmax
                mx = wpool.tile([P, 1], f32, tag="mx")
                nc.vector.tensor_reduce(mx[:], o2b[:], axis=AX, op=OP.max)
                sh = wpool.tile([P, NCLS], f32, tag="sh")
                nc.vector.tensor_scalar(sh[:], o2b[:], mx[:], None, op0=OP.subtract)
                exs = wpool.tile([P, NCLS], f32, tag="exs")
                nc.scalar.activation(exs[:], sh[:], AF.Exp)
                se = wpool.tile([P, 1], f32, tag="se")
                nc.vector.tensor_reduce(se[:], exs[:], axis=AX, op=OP.add)
                lg = wpool.tile([P, 1], f32, tag="lg")
                nc.scalar.activation(lg[:], se[:], AF.Ln)
                fin = wpool.tile([P, NCLS], f32, tag="fin")
                nc.vector.tensor_scalar(fin[:], sh[:], lg[:], None, op0=OP.subtract)
                nc.sync.dma_start(out[t * 128 : t * 128 + P, :], fin[:])

    nc.compile()
    return nc


# --------------------------------------------------------------------------
# Entry point
# --------------------------------------------------------------------------
_last_results = None


def kernel(**inputs):
    global _last_results
    import os

    cfg = _default_cfg()
    meta, in_maps, perm_nodes = host_prep(inputs, cfg)
    nc = build_program(meta)
    from concourse import bass_utils

    trace = os.environ.get("GAT_TRACE") == "1"
    res = bass_utils.run_bass_kernel_spmd(
        nc, in_maps, core_ids=list(range(cfg["NC"])), trace=trace
    )
    _last_results = res
    N, NCLS = cfg["N"], cfg["NCLS"]
    out_full = np.empty((N, NCLS), dtype=np.float32)
    for c in range(cfg["NC"]):
        out_full[perm_nodes[c]] = res.results[c]["out"]
    return out_full
